# revision 14
# baseline (speedup 1.0000x reference)
"""EntityCrossAttention Trainium2 kernel.

Reference computation (per batch b):
    E = noun_feats[class_ids[b]]            [N, D]
    Q = X @ Wq.T + bq                       [T, D]
    K = E @ Wk.T + bk                       [N, D]
    V = E @ Wv.T + bv                       [N, D]
    S = Q @ K.T / sqrt(D)                   [T, N]
    attn = softmax(S, -1)
    wa = attn * w;  wa /= wa.sum(-1) + 1e-6
    out = wa @ V                            [T, D]

Key algebraic restructuring: S = X @ (Wq.T @ K.T) / sqrt(D) + (bq @ K.T)/sqrt(D),
so the [D,D] Q projection never has to be materialized on device. Per batch we
precompute (host, tiny):
    M  = Wq.T @ K.T               [D, N]
    eb = (bq @ K.T) / sqrt(D)     [N]
    V' = w[:,None] * V            [N, D]
    wpe = w + 1e-6                [N]
and the unnormalized weights e = exp(S/sqrt(D)) give
    out = (e @ V') / (e @ wpe)    (exact softmax+reweight+renorm algebra)

X is passed pre-transposed per core (xt [D, rows]) so the device kernel needs
no on-chip transposes. Per 512-row group:
    scoresT = M.T @ Xt   [N, 512]   (PE, f32r: 1 cyc/row at N>=256)
    eT = exp(scoresT*scale + eb)    (ScalarE, PSUM->SBUF)
    per 128-row subtile a:
      den = eT_a.T @ wpe  [128,1] (PE);  raw = eT_a.T @ V'  [128, D] (PE, f32r)
      out_a = raw * (1/den)   (reciprocal on DVE, scaled copy on ScalarE/DVE)

Sharding: data-parallel over B: 8 cores x 2 batches each. Loads go on the SP
HWDGE ring, stores on the ACT HWDGE ring; 1 MiB per DMA. Memory-bound target:
16 MiB in + 16 MiB out per core.
"""

import numpy as np

B, T, D, C, N = 16, 4096, 512, 14, 32
N_CORES = 8
B_PC = B // N_CORES          # batches per core
ROWS_PC = B_PC * T           # 8192
RT = 128                     # row subtile
GR = 256                     # rows per group (one 512 KiB DMA each way)
SH = min(512, GR)            # scores width (PSUM bank / fp32 matmul N limit)
KC = D // 128                # 4 contraction chunks
SCALE = float(D) ** -0.5

# When True, stream X (and M) as bf16: halves the input DMA and doubles the
# scores-matmul stream rate. Output path (exp/V'/denominator) stays f32r.
X_BF16 = False

_compiled = None


def _build():
    import concourse.bacc as bacc
    import concourse.tile as tile
    import concourse.mybir as mybir

    f32 = mybir.dt.float32
    f32r = mybir.dt.float32r
    xdt = mybir.dt.bfloat16 if X_BF16 else f32r
    Exp = mybir.ActivationFunctionType.Exp
    Copy = mybir.ActivationFunctionType.Copy

    nc = bacc.Bacc("TRN2", debug=False)
    x = nc.dram_tensor("x", [D, ROWS_PC], xdt, kind="ExternalInput").ap()
    m = nc.dram_tensor("m", [128, B_PC * KC * N], xdt, kind="ExternalInput").ap()
    vp = nc.dram_tensor("vp", [N, B_PC * D], f32r, kind="ExternalInput").ap()
    wpe = nc.dram_tensor("wpe", [N, 2 * B_PC], f32r, kind="ExternalInput").ap()
    eb = nc.dram_tensor("eb", [N, B_PC], f32, kind="ExternalInput").ap()
    out = nc.dram_tensor("out", [ROWS_PC, D], f32, kind="ExternalOutput").ap()

    x_r = x.rearrange("(k p) r -> p k r", p=128)  # [128, KC, ROWS_PC]

    with tile.TileContext(nc) as tc:
        with (
            tc.tile_pool(name="const", bufs=1) as cpool,
            tc.tile_pool(name="xin", bufs=6) as xpool,
            tc.tile_pool(name="et", bufs=4) as epool,
            tc.tile_pool(name="res", bufs=6) as rpool,
            tc.tile_pool(name="ps_sc", bufs=3, space="PSUM") as ps_sc,
            tc.tile_pool(name="ps_den", bufs=2, space="PSUM") as ps_den,
            tc.tile_pool(name="ps_o", bufs=3, space="PSUM") as ps_o,
        ):
            m_sb = cpool.tile([128, B_PC * KC * N], xdt)
            nc.sync.dma_start(m_sb[:, :], m[:, :])
            vp_sb = cpool.tile([N, B_PC * D], f32r)
            nc.sync.dma_start(vp_sb[:, :], vp[:, :])
            wpe_sb = cpool.tile([N, 2 * B_PC], f32r)
            nc.sync.dma_start(wpe_sb[:, :], wpe[:, :])
            eb_sb = cpool.tile([N, B_PC], f32)
            nc.sync.dma_start(eb_sb[:, :], eb[:, :])

            for b in range(B_PC):
                for g in range(T // GR):
                    r0 = b * T + g * GR
                    # load Xt group [128, KC, GR] on the SP HWDGE ring (1 MiB)
                    x_sb = xpool.tile([128, KC * GR], xdt)
                    nc.sync.dma_start(
                        x_sb[:, :].rearrange("p (k r) -> p k r", k=KC),
                        x_r[:, :, r0 : r0 + GR],
                    )

                    e_sb = epool.tile([N, GR], f32r)
                    for h in range(GR // SH):
                        sc_ps = ps_sc.tile([N, SH], f32)
                        for k in range(KC):
                            nc.tensor.matmul(
                                sc_ps[:, :],
                                m_sb[:, (b * KC + k) * N : (b * KC + k + 1) * N],
                                x_sb[:, k * GR + h * SH : k * GR + (h + 1) * SH],
                                start=(k == 0),
                                stop=(k == KC - 1),
                            )
                        nc.scalar.activation(
                            e_sb[:, h * SH : (h + 1) * SH], sc_ps[:, :], Exp,
                            bias=eb_sb[:, b : b + 1], scale=SCALE,
                        )

                    o_sb = rpool.tile([RT, (GR // RT) * D], f32)
                    for a in range(GR // RT):
                        ea = e_sb[:, a * RT : (a + 1) * RT]
                        den_ps = ps_den.tile([RT, 2], f32)
                        nc.tensor.matmul(
                            den_ps[:, :], ea, wpe_sb[:, 2 * b : 2 * b + 2],
                            start=True, stop=True,
                        )
                        o_ps = ps_o.tile([RT, D], f32)
                        nc.tensor.matmul(
                            o_ps[:, :],
                            ea,
                            vp_sb[:, b * D : (b + 1) * D],
                            start=True, stop=True,
                        )
                        rc_sb = rpool.tile([RT, 1], f32)
                        nc.vector.reciprocal(rc_sb[:, :], den_ps[:, 0:1])
                        if a % 2 == 0:
                            nc.scalar.activation(
                                o_sb[:, a * D : (a + 1) * D], o_ps[:, :], Copy,
                                scale=rc_sb[:, :],
                            )
                        else:
                            nc.vector.tensor_scalar_mul(
                                o_sb[:, a * D : (a + 1) * D], o_ps[:, :],
                                rc_sb[:, :],
                            )
                    # store on the ACT HWDGE ring (1 MiB)
                    nc.scalar.dma_start(
                        out[r0 : r0 + GR, :].rearrange("(a p) d -> p a d", p=RT),
                        o_sb[:, :].rearrange("p (a d) -> p a d", a=GR // RT),
                    )

    nc.compile()
    return nc


def _get_compiled():
    global _compiled
    if _compiled is None:
        _compiled = _build()
    return _compiled


def kernel(
    visual_feat, noun_feats, class_ids, noun_weights,
    Wq, bq, Wk, bk, Wv, bv,
):
    from concourse.bass_utils import run_bass_kernel_spmd

    visual_feat = np.asarray(visual_feat, dtype=np.float32)
    noun_feats = np.asarray(noun_feats, dtype=np.float32)
    class_ids = np.asarray(class_ids)
    noun_weights = np.asarray(noun_weights, dtype=np.float32)
    Wq, bq = np.asarray(Wq, np.float32), np.asarray(bq, np.float32)
    Wk, bk = np.asarray(Wk, np.float32), np.asarray(bk, np.float32)
    Wv, bv = np.asarray(Wv, np.float32), np.asarray(bv, np.float32)

    # Host precompute of tiny per-batch constants (all O(B*N*D)).
    E = noun_feats[class_ids]                       # [B, N, D]
    W = noun_weights[class_ids]                     # [B, N]
    Kb = E @ Wk.T + bk                              # [B, N, D]
    Vb = E @ Wv.T + bv                              # [B, N, D]
    M = np.einsum("jd,bnj->bdn", Wq, Kb)            # [B, D, N] = Wq.T @ Kb.T
    ebias = (Kb @ bq) * SCALE                       # [B, N]
    Vp = W[:, :, None] * Vb                         # [B, N, D]
    wpe = W + 1e-6                                  # [B, N]

    nc = _get_compiled()

    in_maps = []
    for c in range(N_CORES):
        s = slice(c * B_PC, (c + 1) * B_PC)
        # m layout: [128, b*KC*N + k*N + n] = M[b, k*128 + p, n]
        m_c = np.ascontiguousarray(
            M[s].reshape(B_PC, KC, 128, N).transpose(2, 0, 1, 3).reshape(128, -1)
        )
        xt_c = np.ascontiguousarray(
            visual_feat[s].reshape(ROWS_PC, D).T
        )
        if X_BF16:
            import ml_dtypes

            m_c = m_c.astype(ml_dtypes.bfloat16)
            xt_c = xt_c.astype(ml_dtypes.bfloat16)
        in_maps.append(
            {
                "x": xt_c,
                "m": m_c,
                "vp": np.ascontiguousarray(
                    Vp[s].transpose(1, 0, 2).reshape(N, B_PC * D)
                ),
                "wpe": np.ascontiguousarray(np.repeat(wpe[s].T, 2, axis=1)),
                "eb": np.ascontiguousarray(ebias[s].T),
            }
        )

    global _last_in_maps
    _last_in_maps = in_maps
    res = run_bass_kernel_spmd(nc, in_maps, list(range(N_CORES)))
    out = np.empty((B, T, D), dtype=np.float32)
    for c in range(N_CORES):
        out[c * B_PC : (c + 1) * B_PC] = res.results[c]["out"].reshape(B_PC, T, D)
    return out


# revision 15
# speedup vs baseline: 1.0478x; 1.0478x over previous
"""EntityCrossAttention Trainium2 kernel.

Reference computation (per batch b):
    E = noun_feats[class_ids[b]]            [N, D]
    Q = X @ Wq.T + bq                       [T, D]
    K = E @ Wk.T + bk                       [N, D]
    V = E @ Wv.T + bv                       [N, D]
    S = Q @ K.T / sqrt(D)                   [T, N]
    attn = softmax(S, -1)
    wa = attn * w;  wa /= wa.sum(-1) + 1e-6
    out = wa @ V                            [T, D]

Key algebraic restructuring: S = X @ (Wq.T @ K.T) / sqrt(D) + (bq @ K.T)/sqrt(D),
so the [D,D] Q projection never has to be materialized on device. Per batch we
precompute (host, tiny):
    M  = Wq.T @ K.T               [D, N]
    eb = (bq @ K.T) / sqrt(D)     [N]
    V' = w[:,None] * V            [N, D]
    wpe = w + 1e-6                [N]
and the unnormalized weights e = exp(S/sqrt(D)) give
    out = (e @ V') / (e @ wpe)    (exact softmax+reweight+renorm algebra)

X is passed pre-transposed per core (xt [D, rows]) so the device kernel needs
no on-chip transposes. Per 512-row group:
    scoresT = M.T @ Xt   [N, 512]   (PE, f32r: 1 cyc/row at N>=256)
    eT = exp(scoresT*scale + eb)    (ScalarE, PSUM->SBUF)
    per 128-row subtile a:
      den = eT_a.T @ wpe  [128,1] (PE);  raw = eT_a.T @ V'  [128, D] (PE, f32r)
      out_a = raw * (1/den)   (reciprocal on DVE, scaled copy on ScalarE/DVE)

Sharding: data-parallel over B: 8 cores x 2 batches each. Loads go on the SP
HWDGE ring, stores on the ACT HWDGE ring; 1 MiB per DMA. Memory-bound target:
16 MiB in + 16 MiB out per core.
"""

import numpy as np

B, T, D, C, N = 16, 4096, 512, 14, 32
N_CORES = 8
B_PC = B // N_CORES          # batches per core
ROWS_PC = B_PC * T           # 8192
RT = 128                     # row subtile
GR = 512                     # rows per group (one 1 MiB DMA each way)
SH = min(512, GR)            # scores width (PSUM bank / fp32 matmul N limit)
KC = D // 128                # 4 contraction chunks
SCALE = float(D) ** -0.5

# When True, stream X (and M) as bf16: halves the input DMA and doubles the
# scores-matmul stream rate. Output path (exp/V'/denominator) stays f32r.
X_BF16 = False

_compiled = None


def _build():
    import concourse.bacc as bacc
    import concourse.tile as tile
    import concourse.mybir as mybir

    f32 = mybir.dt.float32
    f32r = mybir.dt.float32r
    xdt = mybir.dt.bfloat16 if X_BF16 else f32r
    Exp = mybir.ActivationFunctionType.Exp
    Copy = mybir.ActivationFunctionType.Copy

    nc = bacc.Bacc("TRN2", debug=False)
    x = nc.dram_tensor("x", [D, ROWS_PC], xdt, kind="ExternalInput").ap()
    m = nc.dram_tensor("m", [128, B_PC * KC * N], xdt, kind="ExternalInput").ap()
    vp = nc.dram_tensor("vp", [N, B_PC * D], f32r, kind="ExternalInput").ap()
    wpe = nc.dram_tensor("wpe", [N, 2 * B_PC], f32r, kind="ExternalInput").ap()
    eb = nc.dram_tensor("eb", [N, B_PC], f32, kind="ExternalInput").ap()
    out = nc.dram_tensor("out", [ROWS_PC, D], f32, kind="ExternalOutput").ap()

    x_r = x.rearrange("(k p) r -> p k r", p=128)  # [128, KC, ROWS_PC]

    with tile.TileContext(nc) as tc:
        with (
            tc.tile_pool(name="const", bufs=1) as cpool,
            tc.tile_pool(name="xin", bufs=4) as xpool,
            tc.tile_pool(name="et", bufs=3) as epool,
            tc.tile_pool(name="res", bufs=4) as rpool,
            tc.tile_pool(name="ps_sc", bufs=2, space="PSUM") as ps_sc,
            tc.tile_pool(name="ps_den", bufs=3, space="PSUM") as ps_den,
            tc.tile_pool(name="ps_o", bufs=3, space="PSUM") as ps_o,
        ):
            m_sb = cpool.tile([128, B_PC * KC * N], xdt)
            nc.sync.dma_start(m_sb[:, :], m[:, :])
            vp_sb = cpool.tile([N, B_PC * D], f32r)
            nc.sync.dma_start(vp_sb[:, :], vp[:, :])
            wpe_sb = cpool.tile([N, 2 * B_PC], f32r)
            nc.sync.dma_start(wpe_sb[:, :], wpe[:, :])
            eb_sb = cpool.tile([N, B_PC], f32)
            nc.sync.dma_start(eb_sb[:, :], eb[:, :])

            for b in range(B_PC):
                for g in range(T // GR):
                    r0 = b * T + g * GR
                    # load Xt group [128, KC, GR] on the SP HWDGE ring (1 MiB)
                    x_sb = xpool.tile([128, KC * GR], xdt)
                    nc.sync.dma_start(
                        x_sb[:, :].rearrange("p (k r) -> p k r", k=KC),
                        x_r[:, :, r0 : r0 + GR],
                    )

                    e_sb = epool.tile([N, GR], f32r)
                    for h in range(GR // SH):
                        sc_ps = ps_sc.tile([N, SH], f32)
                        for k in range(KC):
                            nc.tensor.matmul(
                                sc_ps[:, :],
                                m_sb[:, (b * KC + k) * N : (b * KC + k + 1) * N],
                                x_sb[:, k * GR + h * SH : k * GR + (h + 1) * SH],
                                start=(k == 0),
                                stop=(k == KC - 1),
                            )
                        nc.scalar.activation(
                            e_sb[:, h * SH : (h + 1) * SH], sc_ps[:, :], Exp,
                            bias=eb_sb[:, b : b + 1], scale=SCALE,
                        )

                    o_sb = rpool.tile([RT, (GR // RT) * D], f32)
                    for a in range(GR // RT):
                        ea = e_sb[:, a * RT : (a + 1) * RT]
                        den_ps = ps_den.tile([RT, 2], f32)
                        nc.tensor.matmul(
                            den_ps[:, :], ea, wpe_sb[:, 2 * b : 2 * b + 2],
                            start=True, stop=True,
                        )
                        o_ps = ps_o.tile([RT, D], f32)
                        nc.tensor.matmul(
                            o_ps[:, :],
                            ea,
                            vp_sb[:, b * D : (b + 1) * D],
                            start=True, stop=True,
                        )
                        rc_sb = rpool.tile([RT, 1], f32)
                        nc.vector.reciprocal(rc_sb[:, :], den_ps[:, 0:1])
                        if a % 2 == 0:
                            nc.scalar.activation(
                                o_sb[:, a * D : (a + 1) * D], o_ps[:, :], Copy,
                                scale=rc_sb[:, :],
                            )
                        else:
                            nc.vector.tensor_scalar_mul(
                                o_sb[:, a * D : (a + 1) * D], o_ps[:, :],
                                rc_sb[:, :],
                            )
                    # store on the ACT HWDGE ring (1 MiB)
                    nc.scalar.dma_start(
                        out[r0 : r0 + GR, :].rearrange("(a p) d -> p a d", p=RT),
                        o_sb[:, :].rearrange("p (a d) -> p a d", a=GR // RT),
                    )

    nc.compile()
    return nc


def _get_compiled():
    global _compiled
    if _compiled is None:
        _compiled = _build()
    return _compiled


def kernel(
    visual_feat, noun_feats, class_ids, noun_weights,
    Wq, bq, Wk, bk, Wv, bv,
):
    from concourse.bass_utils import run_bass_kernel_spmd

    visual_feat = np.asarray(visual_feat, dtype=np.float32)
    noun_feats = np.asarray(noun_feats, dtype=np.float32)
    class_ids = np.asarray(class_ids)
    noun_weights = np.asarray(noun_weights, dtype=np.float32)
    Wq, bq = np.asarray(Wq, np.float32), np.asarray(bq, np.float32)
    Wk, bk = np.asarray(Wk, np.float32), np.asarray(bk, np.float32)
    Wv, bv = np.asarray(Wv, np.float32), np.asarray(bv, np.float32)

    # Host precompute of tiny per-batch constants (all O(B*N*D)).
    E = noun_feats[class_ids]                       # [B, N, D]
    W = noun_weights[class_ids]                     # [B, N]
    Kb = E @ Wk.T + bk                              # [B, N, D]
    Vb = E @ Wv.T + bv                              # [B, N, D]
    M = np.einsum("jd,bnj->bdn", Wq, Kb)            # [B, D, N] = Wq.T @ Kb.T
    ebias = (Kb @ bq) * SCALE                       # [B, N]
    Vp = W[:, :, None] * Vb                         # [B, N, D]
    wpe = W + 1e-6                                  # [B, N]

    nc = _get_compiled()

    in_maps = []
    for c in range(N_CORES):
        s = slice(c * B_PC, (c + 1) * B_PC)
        # m layout: [128, b*KC*N + k*N + n] = M[b, k*128 + p, n]
        m_c = np.ascontiguousarray(
            M[s].reshape(B_PC, KC, 128, N).transpose(2, 0, 1, 3).reshape(128, -1)
        )
        xt_c = np.ascontiguousarray(
            visual_feat[s].reshape(ROWS_PC, D).T
        )
        if X_BF16:
            import ml_dtypes

            m_c = m_c.astype(ml_dtypes.bfloat16)
            xt_c = xt_c.astype(ml_dtypes.bfloat16)
        in_maps.append(
            {
                "x": xt_c,
                "m": m_c,
                "vp": np.ascontiguousarray(
                    Vp[s].transpose(1, 0, 2).reshape(N, B_PC * D)
                ),
                "wpe": np.ascontiguousarray(np.repeat(wpe[s].T, 2, axis=1)),
                "eb": np.ascontiguousarray(ebias[s].T),
            }
        )

    global _last_in_maps
    _last_in_maps = in_maps
    res = run_bass_kernel_spmd(nc, in_maps, list(range(N_CORES)))
    out = np.empty((B, T, D), dtype=np.float32)
    for c in range(N_CORES):
        out[c * B_PC : (c + 1) * B_PC] = res.results[c]["out"].reshape(B_PC, T, D)
    return out
